# revision 1
# baseline (speedup 1.0000x reference)
"""Block-sparse attention (block-diagonal mask, full-row softmax) on 8 trn2 cores.

Reference semantics (B=1, H=16, S=4096, D=64, BLOCK=64):
    scores  = (Q @ K^T) / 8                     [S, S] per head
    scores *= blockdiag_mask                    (off-block -> 0, NOT -inf)
    weights = softmax(scores, axis=-1)          (over the FULL row)
    out     = weights @ V

Off-block entries contribute exp(0)=1 to the softmax, so for row q in
block b:
    num_q   = sum_{k in b} e_qk v_k - V_bsum(b) + V_total
    denom_q = sum_{k in b} e_qk - 64 + S
    out_q   = num_q / denom_q
Only the diagonal 64x64 blocks are ever materialized.

Sharding: 16 heads over 8 cores -> 2 heads/core, no cross-core comms.

Per-core pipeline (iteration = pair of 128-row chunks; all-bf16 matmuls):
  - fp32 loads are streamed in 8-chunk slabs (HWDGE); GpSimd casts Q/K
    slabs to bf16, DVE casts V.
  - 4 per-chunk PE transposes -> [64(d), 128(s)] into one PSUM bank; one
    DVE copy drops them into a fixed [66, 512] staging tile whose rows
    64:66 hold constant "mask rows" that add -M^2 to every cross-block
    score, so exp underflows to exact 0 off-block.
  - Per chunk: one 66-partition scores matmul -> [128, 128] PSUM (own
    bank - matmul outputs must sit at bank offset 0 on HW); one exp;
    then E^T @ [V|1], a block-diagonal -1 matmul (-bsum), and a rank-1
    [Vtot|S] accumulate num|denom; reciprocal + per-partition scale.
"""

import numpy as np

H, S, D = 16, 4096, 64
HPC = 2  # heads per core
NCORES = 8
CHUNK = 128
NCHUNK = S // CHUNK  # 32
NPAIR = NCHUNK // 2  # 16
NSLAB = 4
SLABC = NCHUNK // NSLAB  # 8 chunks per slab
SCALE = 0.125  # 1/sqrt(D)
MASK_M = 64.0  # M^2*SCALE = 512: exp underflows to exact 0

_CACHE = {}


def _build_bass():
    import concourse.bass as bass
    import concourse.bacc as bacc
    import concourse.tile as tile
    from concourse import mybir
    from concourse.masks import make_identity

    f32 = mybir.dt.float32
    bf16 = mybir.dt.bfloat16
    EXP = mybir.ActivationFunctionType.Exp
    COPY = mybir.ActivationFunctionType.Copy

    nc = bacc.Bacc(
        "TRN2", target_bir_lowering=False, debug=False, num_devices=NCORES
    )
    q_d = nc.dram_tensor("query", [HPC, S, D], f32, kind="ExternalInput")
    k_d = nc.dram_tensor("key", [HPC, S, D], f32, kind="ExternalInput")
    v_d = nc.dram_tensor("value", [HPC, S, D], f32, kind="ExternalInput")
    o_d = nc.dram_tensor("out", [HPC, S, D], f32, kind="ExternalOutput")

    NT = 4  # fixed staging tiles (mask rows written once)

    with tile.TileContext(nc) as tc:
        with (
            tc.tile_pool(name="consts", bufs=1) as consts,
            tc.tile_pool(name="heads", bufs=2) as heads,
            tc.tile_pool(name="work", bufs=6) as work,
            tc.tile_pool(name="vt", bufs=2) as vtp,
            tc.tile_pool(name="ps_t", bufs=2, space="PSUM") as ps_t,
            tc.tile_pool(name="ps_s", bufs=3, space="PSUM") as ps_s,
            tc.tile_pool(name="ps_o", bufs=3, space="PSUM") as ps_o,
        ):
            identb = consts.tile([128, 128], bf16, tag="identb")
            make_identity(nc, identb)
            ones_row = consts.tile([1, 128], bf16, tag="ones_row")
            nc.gpsimd.memset(ones_row, 1.0)
            ones_col = consts.tile([128, 1], f32, tag="ones_col")
            nc.gpsimd.memset(ones_col, 1.0)

            # Block-diagonal -1 (the "-bsum" correction as a matmul weight)
            negblk = consts.tile([128, 128], bf16, tag="negblk")
            nc.gpsimd.memset(negblk, 0.0)
            nc.gpsimd.memset(negblk[0:64, 0:64], -1.0)
            nc.gpsimd.memset(negblk[64:128, 64:128], -1.0)

            # Fixed transpose-staging tiles [66, 512] bf16:
            # cols [Qc 0:128 | Qc1 128:256 | Kc 256:384 | Kc1 384:512],
            # rows 64:66 = mask rows (written once):
            #   Q side: -M where (r + jb) == 1   (jb = 64-col parity)
            #   K side: +M where  r == jb
            tsbs = []
            for i in range(NT):
                t = consts.tile([66, 512], bf16, tag=f"tsb{i}")
                nc.gpsimd.memset(t[64:66, :], 0.0)
                nc.gpsimd.affine_select(
                    out=t[64:66, 0:256].rearrange("p (w b j) -> p w b j", w=2, b=2),
                    in_=t[64:66, 0:256].rearrange("p (w b j) -> p w b j", w=2, b=2),
                    compare_op=mybir.AluOpType.not_equal,
                    fill=-MASK_M,
                    base=-1,
                    pattern=[[0, 2], [1, 2], [0, 64]],
                    channel_multiplier=1,
                )
                nc.gpsimd.affine_select(
                    out=t[64:66, 256:512].rearrange("p (w b j) -> p w b j", w=2, b=2),
                    in_=t[64:66, 256:512].rearrange("p (w b j) -> p w b j", w=2, b=2),
                    compare_op=mybir.AluOpType.not_equal,
                    fill=MASK_M,
                    base=0,
                    pattern=[[0, 2], [-1, 2], [0, 64]],
                    channel_multiplier=1,
                )
                tsbs.append(t)

            for h in range(HPC):
                qh = heads.tile([128, NCHUNK, D], f32, tag="qh")
                kh = heads.tile([128, NCHUNK, D], f32, tag="kh")
                vh = heads.tile([128, NCHUNK, D + 1], f32, tag="vh")
                oh = heads.tile([128, NCHUNK, D], f32, tag="oh")
                qhb = heads.tile([128, NCHUNK, D], bf16, tag="qhb")
                khb = heads.tile([128, NCHUNK, D], bf16, tag="khb")
                vhb = heads.tile([128, NCHUNK, D + 1], bf16, tag="vhb")

                def slab_dma(dst, src, s):
                    nc.sync.dma_start(
                        out=dst[:, s * SLABC : (s + 1) * SLABC, :],
                        in_=src.rearrange("(c p) d -> p c d", p=128)[
                            :, s * SLABC : (s + 1) * SLABC, :
                        ],
                    )

                # V slabs first (Vtot gates the first EV), then Q/K slabs.
                vdst = vh[:, :, 0:D]
                for s in range(NSLAB):
                    slab_dma(vdst, v_d[h], s)
                for s in range(NSLAB):
                    slab_dma(qh, q_d[h], s)
                    slab_dma(kh, kh_d[h] if False else k_d[h], s)

                nc.vector.memset(vh[:, :, D : D + 1], 1.0)

                # slab casts: V on DVE (fast 2x mode), Q/K on GpSimd (idle)
                for s in range(NSLAB):
                    sl = slice(s * SLABC, (s + 1) * SLABC)
                    nc.vector.tensor_copy(out=vhb[:, sl, :], in_=vh[:, sl, :])
                for s in range(NSLAB):
                    sl = slice(s * SLABC, (s + 1) * SLABC)
                    nc.gpsimd.tensor_copy(out=qhb[:, sl, :], in_=qh[:, sl, :])
                    nc.gpsimd.tensor_copy(out=khb[:, sl, :], in_=kh[:, sl, :])

                # V_total colsum -> vtxb [1, D+1] bf16; element D = S exactly.
                # fp32 matmuls on the otherwise-idle PE during the load window.
                vt_ps = ps_s.tile([1, 4, D + 1], f32, tag="ps")
                for m in range(8):
                    nc.tensor.matmul(
                        vt_ps,
                        ones_col,
                        vh[:, 4 * m : 4 * (m + 1), :],
                        start=(m == 0),
                        stop=(m == 7),
                    )
                vt4 = vtp.tile([1, 4 * (D + 1)], f32, tag="vt4")
                nc.scalar.copy(out=vt4, in_=vt_ps.rearrange("p a b -> p (a b)"))
                vt2 = vtp.tile([1, 2 * (D + 1)], f32, tag="vt2")
                nc.vector.tensor_add(
                    vt2,
                    vt4[:, 0 : 2 * (D + 1)],
                    vt4[:, 2 * (D + 1) : 4 * (D + 1)],
                )
                vtxb = vtp.tile([1, D + 1], bf16, tag="vtxb")
                nc.vector.tensor_add(
                    vtxb, vt2[:, 0 : D + 1], vt2[:, D + 1 : 2 * (D + 1)]
                )

                for j in range(NPAIR):
                    c0 = 2 * j
                    # -- 4 per-chunk transposes into one PSUM bank --
                    pt = ps_t.tile([64, 512], bf16, tag="pt")
                    nc.tensor.transpose(pt[:, 0:128], qhb[:, c0, :], identb)
                    nc.tensor.transpose(pt[:, 128:256], qhb[:, c0 + 1, :], identb)
                    nc.tensor.transpose(pt[:, 256:384], khb[:, c0, :], identb)
                    nc.tensor.transpose(pt[:, 384:512], khb[:, c0 + 1, :], identb)
                    tsb = tsbs[j % NT]
                    nc.vector.tensor_copy(out=tsb[0:64, :], in_=pt)

                    for cc in range(2):
                        c = c0 + cc
                        # -- scores S^T[k, q] with mask rows: cross -> -M^2 --
                        pss = ps_s.tile(
                            [128, 128], f32, tag="ps", name=f"ps_{h}_{j}_{cc}"
                        )
                        nc.tensor.matmul(
                            pss,
                            tsb[:, 256 + 128 * cc : 256 + 128 * (cc + 1)],
                            tsb[:, 128 * cc : 128 * (cc + 1)],
                            start=True,
                            stop=True,
                        )
                        # -- E^T = exp(S^T/8): exact 0 on cross quadrants --
                        et = work.tile([128, 128], bf16, tag="et")
                        nc.scalar.activation(out=et, in_=pss, func=EXP, scale=SCALE)

                        # -- num|denom --
                        po = ps_o.tile(
                            [128, D + 1], f32, tag="po", name=f"po_{h}_{j}_{cc}"
                        )
                        nc.tensor.matmul(
                            po, et, vhb[:, c, :], start=True, stop=False
                        )
                        nc.tensor.matmul(
                            po, negblk, vhb[:, c, :], start=False, stop=False
                        )
                        nc.tensor.matmul(
                            po, ones_row, vtxb, start=False, stop=True
                        )

                        # -- out = num * (1/denom) --
                        rcp = work.tile([128, 1], f32, tag="rcp")
                        nc.vector.reciprocal(out=rcp, in_=po[:, D : D + 1])
                        if cc == 0:
                            nc.vector.tensor_scalar_mul(
                                oh[:, c, :], po[:, 0:D], rcp
                            )
                        else:
                            nc.scalar.activation(
                                out=oh[:, c, :], in_=po[:, 0:D], func=COPY,
                                scale=rcp,
                            )

                # stores go out on the Scalar engine's HWDGE queue so the
                # next head's loads are not stuck behind them on SyncE
                for quarter in range(4):
                    hs = slice(quarter * (NCHUNK // 4), (quarter + 1) * (NCHUNK // 4))
                    nc.scalar.dma_start(
                        out=o_d[h].rearrange("(c p) d -> p c d", p=128)[:, hs, :],
                        in_=oh[:, hs, :],
                    )

    nc.compile()
    return nc


def _get_compiled():
    if "nc" not in _CACHE:
        _CACHE["nc"] = _build_bass()
    return _CACHE["nc"]


def make_in_maps(query, key, value):
    q = np.ascontiguousarray(np.asarray(query).reshape(H, S, D), dtype=np.float32)
    k = np.ascontiguousarray(np.asarray(key).reshape(H, S, D), dtype=np.float32)
    v = np.ascontiguousarray(np.asarray(value).reshape(H, S, D), dtype=np.float32)
    in_maps = []
    for i in range(NCORES):
        sl = slice(i * HPC, (i + 1) * HPC)
        in_maps.append(
            {
                "query": np.ascontiguousarray(q[sl]),
                "key": np.ascontiguousarray(k[sl]),
                "value": np.ascontiguousarray(v[sl]),
            }
        )
    return in_maps


def run_spmd(in_maps, **kwargs):
    from concourse.bass_utils import run_bass_kernel_spmd

    nc = _get_compiled()
    return run_bass_kernel_spmd(nc, in_maps, core_ids=list(range(NCORES)), **kwargs)


def assemble(res):
    outs = [res.results[i]["out"] for i in range(NCORES)]
    return np.concatenate(outs, axis=0).reshape(1, H, S, D).astype(np.float32)


def kernel(query: np.ndarray, key: np.ndarray, value: np.ndarray) -> np.ndarray:
    return assemble(run_spmd(make_in_maps(query, key, value)))



# revision 5
# speedup vs baseline: 1.4667x; 1.4667x over previous
"""Block-sparse attention (block-diagonal mask, full-row softmax) on 8 trn2 cores.

Reference semantics (B=1, H=16, S=4096, D=64, BLOCK=64):
    scores  = (Q @ K^T) / 8                     [S, S] per head
    scores *= blockdiag_mask                    (off-block -> 0, NOT -inf)
    weights = softmax(scores, axis=-1)          (over the FULL row)
    out     = weights @ V

Off-block entries contribute exp(0)=1 to the softmax, so for row q in
block b:
    num_q   = sum_{k in b} (e_qk - 1) v_k + V_total
    denom_q = sum_{k in b} e_qk - 64 + S
    out_q   = num_q / denom_q
Only the diagonal 64x64 blocks are ever materialized.

Sharding: 16 heads over 8 cores -> 2 heads/core, no cross-core comms.

Per-core pipeline (vs the previous version):
  - loads split across BOTH HWDGE rings (sync: Q/K, scalar: V+stores) so
    descriptor issue is not single-ring-bound; both heads' DMAs are
    issued up front (all tiles double-buffered) so there is no
    inter-head stall.
  - fp32->bf16 casts on DVE (GpSimd casts measured 5x slower).
  - mask-row trick (rows 64:66 of the staging tiles add -M^2 to
    cross-block scores so exp underflows to exact 0) unchanged.
  - scores matmuls for 4 chunks share one PSUM bank (start=only-first
    clears the bank; later MMs land at fresh offsets) so ONE ACT exp
    instruction covers 4 chunks ([128, 512]: ~(512+352)/1.2 ns).
  - E-1 via one DVE tensor_tensor with a broadcast block-diagonal tile
    (replaces the per-chunk -1 matmul); the +V_total|S correction via
    one DVE broadcast add per quad (replaces the per-chunk rank-1
    matmul); reciprocal batched per quad; final scale one broadcast
    multiply per quad.
"""

import numpy as np

H, S, D = 16, 4096, 64
HPC = 2  # heads per core
NCORES = 8
CHUNK = 128
NCHUNK = S // CHUNK  # 32
NPAIR = NCHUNK // 2  # 16
NQUAD = NCHUNK // 4  # 8
NSLAB = 4
SLABC = NCHUNK // NSLAB  # 8 chunks per slab
SCALE = 0.125  # 1/sqrt(D)
MASK_M = 64.0  # M^2*SCALE = 512: exp underflows to exact 0

_CACHE = {}


def _build_bass():
    import concourse.bass as bass
    import concourse.bacc as bacc
    import concourse.tile as tile
    from concourse import mybir
    from concourse.masks import make_identity

    f32 = mybir.dt.float32
    bf16 = mybir.dt.bfloat16
    EXP = mybir.ActivationFunctionType.Exp

    nc = bacc.Bacc(
        "TRN2", target_bir_lowering=False, debug=False, num_devices=NCORES
    )
    q_d = nc.dram_tensor("query", [HPC, S, D], f32, kind="ExternalInput")
    k_d = nc.dram_tensor("key", [HPC, S, D], f32, kind="ExternalInput")
    v_d = nc.dram_tensor("value", [HPC, S, D], f32, kind="ExternalInput")
    o_d = nc.dram_tensor("out", [HPC, S, D], f32, kind="ExternalOutput")

    NT = 3  # fixed transpose-staging tiles (mask rows written once)

    with tile.TileContext(nc) as tc:
        with (
            tc.tile_pool(name="consts", bufs=1) as consts,
            tc.tile_pool(name="heads", bufs=2) as heads,
            tc.tile_pool(name="work", bufs=3) as work,
            tc.tile_pool(name="norm", bufs=3) as norm,
            tc.tile_pool(name="vt", bufs=2) as vtp,
            tc.tile_pool(name="ps_t", bufs=2, space="PSUM") as ps_t,
            tc.tile_pool(name="ps_s", bufs=2, space="PSUM") as ps_s,
            tc.tile_pool(name="ps_o", bufs=2, space="PSUM") as ps_o,
            tc.tile_pool(name="ps_v", bufs=1, space="PSUM") as ps_v,
        ):
            identb = consts.tile([128, 128], bf16, tag="identb")
            make_identity(nc, identb)
            ones_col = consts.tile([128, 1], bf16, tag="ones_col")
            nc.gpsimd.memset(ones_col, 1.0)
            ones_row = consts.tile([1, 128], bf16, tag="ones_row")
            nc.gpsimd.memset(ones_row, 1.0)

            # Block-diagonal +1 (subtracted from E on DVE)
            blkdiag = consts.tile([128, 128], bf16, tag="blkdiag")
            nc.gpsimd.memset(blkdiag, 0.0)
            nc.gpsimd.memset(blkdiag[0:64, 0:64], 1.0)
            nc.gpsimd.memset(blkdiag[64:128, 64:128], 1.0)

            # Fixed transpose-staging tiles [66, 512] bf16:
            # cols [Qc 0:128 | Qc1 128:256 | Kc 256:384 | Kc1 384:512],
            # rows 64:66 = mask rows (written once):
            #   Q side: -M where (r + jb) == 1   (jb = 64-col parity)
            #   K side: +M where  r == jb
            tsbs = []
            for i in range(NT):
                t = consts.tile([66, 512], bf16, tag=f"tsb{i}")
                nc.gpsimd.memset(t[64:66, :], 0.0)
                nc.gpsimd.affine_select(
                    out=t[64:66, 0:256].rearrange("p (w b j) -> p w b j", w=2, b=2),
                    in_=t[64:66, 0:256].rearrange("p (w b j) -> p w b j", w=2, b=2),
                    compare_op=mybir.AluOpType.not_equal,
                    fill=-MASK_M,
                    base=-1,
                    pattern=[[0, 2], [1, 2], [0, 64]],
                    channel_multiplier=1,
                )
                nc.gpsimd.affine_select(
                    out=t[64:66, 256:512].rearrange("p (w b j) -> p w b j", w=2, b=2),
                    in_=t[64:66, 256:512].rearrange("p (w b j) -> p w b j", w=2, b=2),
                    compare_op=mybir.AluOpType.not_equal,
                    fill=MASK_M,
                    base=0,
                    pattern=[[0, 2], [-1, 2], [0, 64]],
                    channel_multiplier=1,
                )
                tsbs.append(t)

            for h in range(HPC):
                qh = heads.tile([128, NCHUNK, D], f32, tag="qh")
                kh = heads.tile([128, NCHUNK, D], f32, tag="kh")
                vh = heads.tile([128, NCHUNK, D + 1], f32, tag="vh")
                oh = heads.tile([128, NCHUNK, D], f32, tag="oh")
                qhb = heads.tile([128, NCHUNK, D], bf16, tag="qhb")
                khb = heads.tile([128, NCHUNK, D], bf16, tag="khb")
                vhb = heads.tile([128, NCHUNK, D + 1], bf16, tag="vhb")

                def slab_dma(eng, dst, src, s):
                    eng.dma_start(
                        out=dst[:, s * SLABC : (s + 1) * SLABC, :],
                        in_=src.rearrange("(c p) d -> p c d", p=128)[
                            :, s * SLABC : (s + 1) * SLABC, :
                        ],
                    )

                # V on the scalar HWDGE ring, Q/K interleaved on sync ring.
                vdst = vh[:, :, 0:D]
                for s in range(NSLAB):
                    slab_dma(nc.scalar, vdst, v_d[h], s)
                for s in range(NSLAB):
                    slab_dma(nc.sync, qh, q_d[h], s)
                    slab_dma(nc.sync, kh, k_d[h], s)

                nc.vector.memset(vh[:, :, D : D + 1], 1.0)

                # slab casts on DVE
                for s in range(NSLAB):
                    sl = slice(s * SLABC, (s + 1) * SLABC)
                    nc.vector.tensor_copy(out=vhb[:, sl, :], in_=vh[:, sl, :])
                    nc.vector.tensor_copy(out=qhb[:, sl, :], in_=qh[:, sl, :])
                    nc.vector.tensor_copy(out=khb[:, sl, :], in_=kh[:, sl, :])

                # V_total colsum: accumulate 8 half-slab matmuls into one
                # [1, 4, 65] PSUM window; DVE tree-add -> vtx [1, 65]
                # (col D = S = 4096 exactly since vh col D is all-ones);
                # then rank-1 broadcast to vtotbc [128, 65] f32 in SBUF.
                vt_ps = ps_v.tile([1, 4, D + 1], f32, tag="vt_ps")
                for s in range(2 * NSLAB):
                    nc.tensor.matmul(
                        vt_ps,
                        ones_col,
                        vhb[:, 4 * s : 4 * (s + 1), :],
                        start=(s == 0),
                        stop=(s == 2 * NSLAB - 1),
                    )
                vt4 = vtp.tile([1, 4, D + 1], f32, tag="vt4")
                nc.vector.tensor_copy(out=vt4, in_=vt_ps)
                vt2 = vtp.tile([1, 2, D + 1], f32, tag="vt2")
                nc.vector.tensor_add(vt2, vt4[:, 0:2, :], vt4[:, 2:4, :])
                vtxb = vtp.tile([1, D + 1], bf16, tag="vtxb")
                nc.vector.tensor_add(vtxb, vt2[:, 0, :], vt2[:, 1, :])
                vbc_ps = ps_v.tile([128, D + 1], f32, tag="vbc_ps")
                nc.tensor.matmul(vbc_ps, ones_row, vtxb, start=True, stop=True)
                vtotbc = vtp.tile([128, D + 1], f32, tag="vtotbc")
                nc.vector.tensor_copy(out=vtotbc, in_=vbc_ps)

                for g in range(NQUAD):
                    # 4-chunk quad: 2 transpose pairs + 4 scores MMs into
                    # one PSUM bank, one exp, one sub, 4 po MMs into one
                    # bank, one add, one rcp, one scale.
                    pss = ps_s.tile([128, 4, 128], f32, tag="pss")
                    for j in range(2):
                        c0 = 4 * g + 2 * j
                        pt = ps_t.tile([64, 512], bf16, tag="pt")
                        nc.tensor.transpose(pt[:, 0:128], qhb[:, c0, :], identb)
                        nc.tensor.transpose(
                            pt[:, 128:256], qhb[:, c0 + 1, :], identb
                        )
                        nc.tensor.transpose(
                            pt[:, 256:384], khb[:, c0, :], identb
                        )
                        nc.tensor.transpose(
                            pt[:, 384:512], khb[:, c0 + 1, :], identb
                        )
                        tsb = tsbs[(2 * g + j) % NT]
                        nc.vector.tensor_copy(out=tsb[0:64, :], in_=pt)

                        for cc in range(2):
                            qi = 2 * j + cc
                            nc.tensor.matmul(
                                pss[:, qi, :],
                                tsb[:, 256 + 128 * cc : 256 + 128 * (cc + 1)],
                                tsb[:, 128 * cc : 128 * (cc + 1)],
                                start=(qi == 0),
                                stop=(qi == 3),
                            )

                    # E^T = exp(S^T/8) for 4 chunks in one ACT op
                    et = work.tile([128, 4, 128], bf16, tag="et")
                    nc.scalar.activation(out=et, in_=pss, func=EXP, scale=SCALE)
                    # E^T - blockdiag(1): one DVE op, broadcast in1
                    etm = work.tile([128, 4, 128], bf16, tag="etm")
                    nc.vector.tensor_sub(
                        etm,
                        et,
                        blkdiag[:].unsqueeze(1).broadcast_to((128, 4, 128)),
                    )

                    # num|denom partials: po = (E-1)^T @ [V|1]
                    po = ps_o.tile([128, 4, D + 1], f32, tag="po")
                    for qi in range(4):
                        c = 4 * g + qi
                        nc.tensor.matmul(
                            po[:, qi, :],
                            etm[:, qi, :],
                            vhb[:, c, :],
                            start=(qi == 0),
                            stop=(qi == 3),
                        )

                    # tq = po + [Vtot | S]  (broadcast add over the quad)
                    tq = norm.tile([128, 4, D + 1], f32, tag="tq")
                    nc.vector.tensor_add(
                        tq,
                        po,
                        vtotbc[:].unsqueeze(1).broadcast_to((128, 4, D + 1)),
                    )
                    # rcp = 1/denom for 4 chunks at once
                    rr = norm.tile([128, 4], f32, tag="rr")
                    nc.vector.reciprocal(out=rr, in_=tq[:, :, D])
                    # out = num * rcp (broadcast multiply)
                    nc.vector.tensor_mul(
                        oh[:, 4 * g : 4 * g + 4, :],
                        tq[:, :, 0:D],
                        rr[:].unsqueeze(2).broadcast_to((128, 4, D)),
                    )

                # stores on the scalar HWDGE ring, per quarter so they
                # drain during compute
                for quarter in range(4):
                    hs = slice(
                        quarter * (NCHUNK // 4), (quarter + 1) * (NCHUNK // 4)
                    )
                    nc.scalar.dma_start(
                        out=o_d[h].rearrange("(c p) d -> p c d", p=128)[:, hs, :],
                        in_=oh[:, hs, :],
                    )

    nc.compile()
    return nc


def _get_compiled():
    if "nc" not in _CACHE:
        _CACHE["nc"] = _build_bass()
    return _CACHE["nc"]


def make_in_maps(query, key, value):
    q = np.ascontiguousarray(np.asarray(query).reshape(H, S, D), dtype=np.float32)
    k = np.ascontiguousarray(np.asarray(key).reshape(H, S, D), dtype=np.float32)
    v = np.ascontiguousarray(np.asarray(value).reshape(H, S, D), dtype=np.float32)
    in_maps = []
    for i in range(NCORES):
        sl = slice(i * HPC, (i + 1) * HPC)
        in_maps.append(
            {
                "query": np.ascontiguousarray(q[sl]),
                "key": np.ascontiguousarray(k[sl]),
                "value": np.ascontiguousarray(v[sl]),
            }
        )
    return in_maps


def run_spmd(in_maps, **kwargs):
    from concourse.bass_utils import run_bass_kernel_spmd

    nc = _get_compiled()
    return run_bass_kernel_spmd(nc, in_maps, core_ids=list(range(NCORES)), **kwargs)


def assemble(res):
    outs = [res.results[i]["out"] for i in range(NCORES)]
    return np.concatenate(outs, axis=0).reshape(1, H, S, D).astype(np.float32)


def kernel(query: np.ndarray, key: np.ndarray, value: np.ndarray) -> np.ndarray:
    return assemble(run_spmd(make_in_maps(query, key, value)))
